# revision 37
# baseline (speedup 1.0000x reference)
"""Trainium2 Bass kernel for nn_AttentionMatrix.

Computes, for mat_0:[B,N,H], mat_1:[B,M,H], w:[3H], bias:[1]:
    out[b,n,m] = sum_h mat_0[b,n,h]*w2[h]*mat_1[b,m,h] + s0[b,n] + s1[b,m] + C
with s0 = mat_0@w0, s1 = mat_1@w1, C = bias[0].

Strategy: data-parallel over batch across 8 NeuronCores (2 batches/core).
The rank-1 epilogue vectors s0/s1 (0.1% of the FLOPs) are precomputed on
host and passed as derived inputs; the 68.7 GFLOP batched einsum runs on
the PE array in float32r (full rate at 512-wide moving dim).

Per core, per batch:
  - DMA mat_0/mat_1 in natural [n,h] layout (contiguous 1MB loads).
  - PE-transpose both to [h,n]/[h,m] (128x128 blocks, 4 packed per PSUM
    bank), evicted by ACT; mat_0 side scaled by w2 (per-partition scale).
  - mains: psum[128n, 1024m] = sum_k at_k[h,n].T @ bt_k[h,m] (f32r).
  - fused DVE epilogue: out_sbuf = (psum + s0_col) + s1_bcast_row.
  - 2MB contiguous output stores.
"""

import numpy as np

import concourse.bacc as bacc
import concourse.bass as bass
import concourse.mybir as mybir
from concourse.masks import make_identity
from concourse.tile import TileContext

F32 = mybir.dt.float32
F32R = mybir.dt.float32r
ADD = mybir.AluOpType.add
COPY = mybir.ActivationFunctionType.Copy

P = 128

# Problem dims (hardcoded per contract)
B, N, M, H = 16, 2048, 2048, 512
N_CORES = 8
BPC = B // N_CORES  # batches per core


def build_program(bpc=BPC, n=N, m=M, h=H):
    kt = h // P        # contraction k-tiles
    nt = n // P        # n-tiles
    ng = nt // 4       # transpose eviction groups (4 n-tiles each)
    nl = n // 256      # natural-layout load tiles (256 rows each)
    ow = min(1024, m)  # psum main tile width (<= 2 banks)
    sw = 2 if nt % 2 == 0 else 1  # n-strips per output DMA

    nc = bacc.Bacc("TRN2", target_bir_lowering=False, debug=False)
    m0 = nc.dram_tensor("mat_0", [bpc, n, h], F32, kind="ExternalInput").ap()
    m1 = nc.dram_tensor("mat_1", [bpc, m, h], F32, kind="ExternalInput").ap()
    # derived inputs (host-precomputed)
    w2c = nc.dram_tensor("w2c", [P, kt], F32, kind="ExternalInput").ap()
    s0t = nc.dram_tensor("s0t", [bpc, P, nt], F32, kind="ExternalInput").ap()
    s1t = nc.dram_tensor("s1t", [bpc, P, m], F32, kind="ExternalInput").ap()
    out = nc.dram_tensor("out", [bpc, n, m], F32, kind="ExternalOutput").ap()

    with TileContext(nc) as tc:
        with (
            tc.tile_pool(name="const", bufs=1) as cpool,
            tc.tile_pool(name="io", bufs=4) as iopool,
            tc.tile_pool(name="opnd", bufs=1) as tpool,
            tc.tile_pool(name="vecs", bufs=2) as vpool,
            tc.tile_pool(name="ob", bufs=2) as obpool,
            tc.tile_pool(name="mpsum", bufs=2, space="PSUM") as mpsum,
            tc.tile_pool(name="tpsum", bufs=2, space="PSUM") as tpsum,
        ):
            identity = cpool.tile([P, P], F32)
            make_identity(nc, identity)
            w2_cols = cpool.tile([P, kt], F32)
            nc.sync.dma_start(out=w2_cols, in_=w2c)

            def emit_loads(bi):
                anat, bnat = {}, {}
                for l in range(nl):
                    r0 = 256 * l
                    b_t = iopool.tile([P, 2 * h], F32, tag="bnat", name="b_t")
                    nc.sync.dma_start(
                        out=b_t.rearrange("p (t h) -> p t h", t=2),
                        in_=m1[bi, r0:r0 + 256, :].rearrange(
                            "(t p) h -> p t h", p=P
                        ),
                    )
                    bnat[l] = b_t
                for l in range(nl):
                    r0 = 256 * l
                    a_t = iopool.tile([P, 2 * h], F32, tag="anat", name="a_t")
                    nc.sync.dma_start(
                        out=a_t.rearrange("p (t h) -> p t h", t=2),
                        in_=m0[bi, r0:r0 + 256, :].rearrange(
                            "(t p) h -> p t h", p=P
                        ),
                    )
                    anat[l] = a_t
                s0c = vpool.tile([P, nt], F32, tag="s0c", name="s0c")
                nc.sync.dma_start(out=s0c, in_=s0t[bi])
                s1b = vpool.tile([P, m], F32, tag="s1b", name="s1b")
                nc.sync.dma_start(out=s1b, in_=s1t[bi])
                return anat, bnat, s0c, s1b

            def emit_prep(bi, anat, bnat):
                at = [
                    tpool.tile([P, n], F32R, tag=f"at{k}", name=f"at{k}")
                    for k in range(kt)
                ]
                bt = [
                    tpool.tile([P, m], F32R, tag=f"bt{k}", name=f"bt{k}")
                    for k in range(kt)
                ]
                for g in range(ng):
                    for k in range(kt):
                        pt = tpsum.tile([P, 512], F32, tag="tp", name="pt")
                        for j in range(4):
                            t = 4 * g + j
                            src = bnat[t // 2][
                                :, (t % 2) * h + k * P:(t % 2) * h + (k + 1) * P
                            ]
                            nc.tensor.transpose(
                                pt[:, j * P:(j + 1) * P], src, identity
                            )
                        nc.scalar.copy(bt[k][:, g * 512:(g + 1) * 512], pt)
                    for k in range(kt):
                        pt = tpsum.tile([P, 512], F32, tag="tp", name="pt")
                        for j in range(4):
                            t = 4 * g + j
                            src = anat[t // 2][
                                :, (t % 2) * h + k * P:(t % 2) * h + (k + 1) * P
                            ]
                            nc.tensor.transpose(
                                pt[:, j * P:(j + 1) * P], src, identity
                            )
                        nc.scalar.activation(
                            at[k][:, g * 512:(g + 1) * 512],
                            pt,
                            COPY,
                            bias=0.0,
                            scale=w2_cols[:, k:k + 1],
                        )
                return at, bt

            def emit_mains(bi, at, bt, s0c, s1b):
                ob = None
                for t in range(nt):
                    strip = t % sw
                    if strip == 0:
                        ob = obpool.tile([P, sw * m], F32, tag="ob", name="ob", bufs=3)
                    for pc in range(m // ow):
                        mp = mpsum.tile([P, ow], F32, tag="mm", name="mp")
                        for k in range(kt):
                            for mh in range(ow // 512):
                                cm = pc * (ow // 512) + mh
                                nc.tensor.matmul(
                                    mp[:, mh * 512:(mh + 1) * 512],
                                    lhsT=at[k][:, t * P:(t + 1) * P],
                                    rhs=bt[k][:, cm * 512:(cm + 1) * 512],
                                    start=(k == 0),
                                    stop=(k == kt - 1),
                                )
                        nc.vector.scalar_tensor_tensor(
                            out=ob[:, strip * m + pc * ow:strip * m + (pc + 1) * ow],
                            in0=mp,
                            scalar=s0c[:, t:t + 1],
                            in1=s1b[:, pc * ow:(pc + 1) * ow],
                            op0=ADD,
                            op1=ADD,
                        )
                    if bi == bpc - 1 and t >= nt - sw:
                        # final pair: per-strip 1MB stores (shorter tail)
                        nc.sync.dma_start(
                            out=out[bi, t * P:(t + 1) * P, :],
                            in_=ob[:, strip * m:(strip + 1) * m],
                        )
                    elif strip == sw - 1:
                        r0 = (t - sw + 1) * P
                        nc.sync.dma_start(
                            out=out[bi, r0:r0 + sw * P, :].rearrange(
                                "(s p) m -> p s m", p=P
                            ),
                            in_=ob.rearrange("p (s m) -> p s m", s=sw),
                        )

            # software-pipelined emission: next batch's loads go out before
            # this batch's mains so input DMA fills the store-idle window
            la = emit_loads(0)
            prep = emit_prep(0, la[0], la[1])
            vecs = (la[2], la[3])
            for bi in range(1, bpc):
                la_next = emit_loads(bi)
                emit_mains(bi - 1, prep[0], prep[1], vecs[0], vecs[1])
                prep = emit_prep(bi, la_next[0], la_next[1])
                vecs = (la_next[2], la_next[3])
            emit_mains(bpc - 1, prep[0], prep[1], vecs[0], vecs[1])
    nc.compile()
    return nc


_CACHE = {}


def _get_program():
    if "nc" not in _CACHE:
        _CACHE["nc"] = build_program()
    return _CACHE["nc"]


def make_in_maps(inputs, bpc=BPC, n_cores=N_CORES, n=N, m=M, h=H):
    mat_0 = np.ascontiguousarray(np.asarray(inputs["mat_0"], dtype=np.float32))
    mat_1 = np.ascontiguousarray(np.asarray(inputs["mat_1"], dtype=np.float32))
    w = np.asarray(inputs["w"], dtype=np.float32)
    bias = np.asarray(inputs["bias"], dtype=np.float32)
    w0, w1, w2 = w[:h], w[h:2 * h], w[2 * h:]
    kt, nt = h // P, n // P
    # host-side rank-1 epilogue vectors
    s0 = mat_0 @ w0                      # [B, n]
    s1 = mat_1 @ w1 + bias[0]            # [B, m]
    # layouts for direct DMA
    w2c = np.ascontiguousarray(w2.reshape(kt, P).T)          # [P, kt]
    s0t = np.ascontiguousarray(
        s0.reshape(-1, nt, P).transpose(0, 2, 1)             # [B, P, nt]
    )
    s1t = np.ascontiguousarray(
        np.broadcast_to(s1[:, None, :], (s1.shape[0], P, m))  # [B, P, m]
    )
    in_maps = []
    for c in range(n_cores):
        sl = slice(c * bpc, (c + 1) * bpc)
        in_maps.append(
            {
                "mat_0": mat_0[sl],
                "mat_1": mat_1[sl],
                "w2c": w2c,
                "s0t": s0t[sl],
                "s1t": s1t[sl],
            }
        )
    return in_maps


def kernel(**inputs) -> np.ndarray:
    from concourse import bass_utils

    nc = _get_program()
    res = bass_utils.run_bass_kernel_spmd(
        nc, make_in_maps(inputs), core_ids=list(range(N_CORES))
    )
    return np.concatenate(
        [res.results[c]["out"] for c in range(N_CORES)], axis=0
    )


# revision 38
# speedup vs baseline: 1.0036x; 1.0036x over previous
"""Trainium2 Bass kernel for nn_AttentionMatrix.

Computes, for mat_0:[B,N,H], mat_1:[B,M,H], w:[3H], bias:[1]:
    out[b,n,m] = sum_h mat_0[b,n,h]*w2[h]*mat_1[b,m,h] + s0[b,n] + s1[b,m] + C
with s0 = mat_0@w0, s1 = mat_1@w1, C = bias[0].

Strategy: data-parallel over batch across 8 NeuronCores (2 batches/core).
The rank-1 epilogue vectors s0/s1 (0.1% of the FLOPs) are precomputed on
host and passed as derived inputs; the 68.7 GFLOP batched einsum runs on
the PE array in float32r (full rate at 512-wide moving dim).

Per core, per batch:
  - DMA mat_0/mat_1 in natural [n,h] layout (contiguous 1MB loads).
  - PE-transpose both to [h,n]/[h,m] (128x128 blocks, 4 packed per PSUM
    bank), evicted by ACT; mat_0 side scaled by w2 (per-partition scale).
  - mains: psum[128n, 1024m] = sum_k at_k[h,n].T @ bt_k[h,m] (f32r).
  - fused DVE epilogue: out_sbuf = (psum + s0_col) + s1_bcast_row.
  - 2MB contiguous output stores.
"""

import numpy as np

import concourse.bacc as bacc
import concourse.bass as bass
import concourse.mybir as mybir
from concourse.masks import make_identity
from concourse.tile import TileContext

F32 = mybir.dt.float32
F32R = mybir.dt.float32r
ADD = mybir.AluOpType.add
COPY = mybir.ActivationFunctionType.Copy

P = 128

# Problem dims (hardcoded per contract)
B, N, M, H = 16, 2048, 2048, 512
N_CORES = 8
BPC = B // N_CORES  # batches per core


def build_program(bpc=BPC, n=N, m=M, h=H):
    kt = h // P        # contraction k-tiles
    nt = n // P        # n-tiles
    ng = nt // 4       # transpose eviction groups (4 n-tiles each)
    nl = n // 256      # natural-layout load tiles (256 rows each)
    ow = min(1024, m)  # psum main tile width (<= 2 banks)
    sw = 2 if nt % 2 == 0 else 1  # n-strips per output DMA

    nc = bacc.Bacc("TRN2", target_bir_lowering=False, debug=False)
    m0 = nc.dram_tensor("mat_0", [bpc, n, h], F32, kind="ExternalInput").ap()
    m1 = nc.dram_tensor("mat_1", [bpc, m, h], F32, kind="ExternalInput").ap()
    # derived inputs (host-precomputed)
    w2c = nc.dram_tensor("w2c", [P, kt], F32, kind="ExternalInput").ap()
    s0t = nc.dram_tensor("s0t", [bpc, P, nt], F32, kind="ExternalInput").ap()
    s1t = nc.dram_tensor("s1t", [bpc, P, m], F32, kind="ExternalInput").ap()
    out = nc.dram_tensor("out", [bpc, n, m], F32, kind="ExternalOutput").ap()

    with TileContext(nc) as tc:
        with (
            tc.tile_pool(name="const", bufs=1) as cpool,
            tc.tile_pool(name="io", bufs=4) as iopool,
            tc.tile_pool(name="opnd", bufs=1) as tpool,
            tc.tile_pool(name="vecs", bufs=2) as vpool,
            tc.tile_pool(name="ob", bufs=2) as obpool,
            tc.tile_pool(name="mpsum", bufs=2, space="PSUM") as mpsum,
            tc.tile_pool(name="tpsum", bufs=2, space="PSUM") as tpsum,
        ):
            identity = cpool.tile([P, P], F32)
            make_identity(nc, identity)
            w2_cols = cpool.tile([P, kt], F32)

            def emit_loads(bi):
                anat, bnat = {}, {}
                for l in range(nl):
                    r0 = 256 * l
                    b_t = iopool.tile([P, 2 * h], F32, tag="bnat", name="b_t")
                    nc.sync.dma_start(
                        out=b_t.rearrange("p (t h) -> p t h", t=2),
                        in_=m1[bi, r0:r0 + 256, :].rearrange(
                            "(t p) h -> p t h", p=P
                        ),
                    )
                    bnat[l] = b_t
                    if bi == 0 and l == 0:
                        # w2_cols only gates the first A-eviction (~10us in);
                        # keep it off the FIFO head so B loads start at t=0
                        nc.sync.dma_start(out=w2_cols, in_=w2c)
                for l in range(nl):
                    r0 = 256 * l
                    a_t = iopool.tile([P, 2 * h], F32, tag="anat", name="a_t")
                    nc.sync.dma_start(
                        out=a_t.rearrange("p (t h) -> p t h", t=2),
                        in_=m0[bi, r0:r0 + 256, :].rearrange(
                            "(t p) h -> p t h", p=P
                        ),
                    )
                    anat[l] = a_t
                s0c = vpool.tile([P, nt], F32, tag="s0c", name="s0c")
                nc.sync.dma_start(out=s0c, in_=s0t[bi])
                s1b = vpool.tile([P, m], F32, tag="s1b", name="s1b")
                nc.sync.dma_start(out=s1b, in_=s1t[bi])
                return anat, bnat, s0c, s1b

            def emit_prep(bi, anat, bnat):
                at = [
                    tpool.tile([P, n], F32R, tag=f"at{k}", name=f"at{k}")
                    for k in range(kt)
                ]
                bt = [
                    tpool.tile([P, m], F32R, tag=f"bt{k}", name=f"bt{k}")
                    for k in range(kt)
                ]
                for g in range(ng):
                    for k in range(kt):
                        pt = tpsum.tile([P, 512], F32, tag="tp", name="pt")
                        for j in range(4):
                            t = 4 * g + j
                            src = bnat[t // 2][
                                :, (t % 2) * h + k * P:(t % 2) * h + (k + 1) * P
                            ]
                            nc.tensor.transpose(
                                pt[:, j * P:(j + 1) * P], src, identity
                            )
                        nc.scalar.copy(bt[k][:, g * 512:(g + 1) * 512], pt)
                    for k in range(kt):
                        pt = tpsum.tile([P, 512], F32, tag="tp", name="pt")
                        for j in range(4):
                            t = 4 * g + j
                            src = anat[t // 2][
                                :, (t % 2) * h + k * P:(t % 2) * h + (k + 1) * P
                            ]
                            nc.tensor.transpose(
                                pt[:, j * P:(j + 1) * P], src, identity
                            )
                        nc.scalar.activation(
                            at[k][:, g * 512:(g + 1) * 512],
                            pt,
                            COPY,
                            bias=0.0,
                            scale=w2_cols[:, k:k + 1],
                        )
                return at, bt

            def emit_mains(bi, at, bt, s0c, s1b):
                ob = None
                for t in range(nt):
                    strip = t % sw
                    if strip == 0:
                        ob = obpool.tile([P, sw * m], F32, tag="ob", name="ob", bufs=3)
                    for pc in range(m // ow):
                        mp = mpsum.tile([P, ow], F32, tag="mm", name="mp")
                        for k in range(kt):
                            for mh in range(ow // 512):
                                cm = pc * (ow // 512) + mh
                                nc.tensor.matmul(
                                    mp[:, mh * 512:(mh + 1) * 512],
                                    lhsT=at[k][:, t * P:(t + 1) * P],
                                    rhs=bt[k][:, cm * 512:(cm + 1) * 512],
                                    start=(k == 0),
                                    stop=(k == kt - 1),
                                )
                        nc.vector.scalar_tensor_tensor(
                            out=ob[:, strip * m + pc * ow:strip * m + (pc + 1) * ow],
                            in0=mp,
                            scalar=s0c[:, t:t + 1],
                            in1=s1b[:, pc * ow:(pc + 1) * ow],
                            op0=ADD,
                            op1=ADD,
                        )
                    if bi == bpc - 1 and t >= nt - sw:
                        # final pair: per-strip 1MB stores (shorter tail)
                        nc.sync.dma_start(
                            out=out[bi, t * P:(t + 1) * P, :],
                            in_=ob[:, strip * m:(strip + 1) * m],
                        )
                    elif strip == sw - 1:
                        r0 = (t - sw + 1) * P
                        nc.sync.dma_start(
                            out=out[bi, r0:r0 + sw * P, :].rearrange(
                                "(s p) m -> p s m", p=P
                            ),
                            in_=ob.rearrange("p (s m) -> p s m", s=sw),
                        )

            # software-pipelined emission: next batch's loads go out before
            # this batch's mains so input DMA fills the store-idle window
            la = emit_loads(0)
            prep = emit_prep(0, la[0], la[1])
            vecs = (la[2], la[3])
            for bi in range(1, bpc):
                la_next = emit_loads(bi)
                emit_mains(bi - 1, prep[0], prep[1], vecs[0], vecs[1])
                prep = emit_prep(bi, la_next[0], la_next[1])
                vecs = (la_next[2], la_next[3])
            emit_mains(bpc - 1, prep[0], prep[1], vecs[0], vecs[1])
    nc.compile()
    return nc


_CACHE = {}


def _get_program():
    if "nc" not in _CACHE:
        _CACHE["nc"] = build_program()
    return _CACHE["nc"]


def make_in_maps(inputs, bpc=BPC, n_cores=N_CORES, n=N, m=M, h=H):
    mat_0 = np.ascontiguousarray(np.asarray(inputs["mat_0"], dtype=np.float32))
    mat_1 = np.ascontiguousarray(np.asarray(inputs["mat_1"], dtype=np.float32))
    w = np.asarray(inputs["w"], dtype=np.float32)
    bias = np.asarray(inputs["bias"], dtype=np.float32)
    w0, w1, w2 = w[:h], w[h:2 * h], w[2 * h:]
    kt, nt = h // P, n // P
    # host-side rank-1 epilogue vectors
    s0 = mat_0 @ w0                      # [B, n]
    s1 = mat_1 @ w1 + bias[0]            # [B, m]
    # layouts for direct DMA
    w2c = np.ascontiguousarray(w2.reshape(kt, P).T)          # [P, kt]
    s0t = np.ascontiguousarray(
        s0.reshape(-1, nt, P).transpose(0, 2, 1)             # [B, P, nt]
    )
    s1t = np.ascontiguousarray(
        np.broadcast_to(s1[:, None, :], (s1.shape[0], P, m))  # [B, P, m]
    )
    in_maps = []
    for c in range(n_cores):
        sl = slice(c * bpc, (c + 1) * bpc)
        in_maps.append(
            {
                "mat_0": mat_0[sl],
                "mat_1": mat_1[sl],
                "w2c": w2c,
                "s0t": s0t[sl],
                "s1t": s1t[sl],
            }
        )
    return in_maps


def kernel(**inputs) -> np.ndarray:
    from concourse import bass_utils

    nc = _get_program()
    res = bass_utils.run_bass_kernel_spmd(
        nc, make_in_maps(inputs), core_ids=list(range(N_CORES))
    )
    return np.concatenate(
        [res.results[c]["out"] for c in range(N_CORES)], axis=0
    )


# revision 39
# speedup vs baseline: 1.0197x; 1.0161x over previous
"""Trainium2 Bass kernel for nn_AttentionMatrix.

Computes, for mat_0:[B,N,H], mat_1:[B,M,H], w:[3H], bias:[1]:
    out[b,n,m] = sum_h mat_0[b,n,h]*w2[h]*mat_1[b,m,h] + s0[b,n] + s1[b,m] + C
with s0 = mat_0@w0, s1 = mat_1@w1, C = bias[0].

Strategy: data-parallel over batch across 8 NeuronCores (2 batches/core).
The rank-1 epilogue vectors s0/s1 (0.1% of the FLOPs) are precomputed on
host and passed as derived inputs; the 68.7 GFLOP batched einsum runs on
the PE array in float32r (full rate at 512-wide moving dim).

Per core, per batch:
  - DMA mat_0/mat_1 in natural [n,h] layout (contiguous 1MB loads).
  - PE-transpose both to [h,n]/[h,m] (128x128 blocks, 4 packed per PSUM
    bank), evicted by ACT; mat_0 side scaled by w2 (per-partition scale).
  - mains: psum[128n, 1024m] = sum_k at_k[h,n].T @ bt_k[h,m] (f32r).
  - fused DVE epilogue: out_sbuf = (psum + s0_col) + s1_bcast_row.
  - 2MB contiguous output stores.
"""

import numpy as np

import concourse.bacc as bacc
import concourse.bass as bass
import concourse.mybir as mybir
from concourse.masks import make_identity
from concourse.tile import TileContext

F32 = mybir.dt.float32
F32R = mybir.dt.float32r
ADD = mybir.AluOpType.add
COPY = mybir.ActivationFunctionType.Copy

P = 128

# Problem dims (hardcoded per contract)
B, N, M, H = 16, 2048, 2048, 512
N_CORES = 8
BPC = B // N_CORES  # batches per core


def build_program(bpc=BPC, n=N, m=M, h=H):
    kt = h // P        # contraction k-tiles
    nt = n // P        # n-tiles
    ng = nt // 4       # transpose eviction groups (4 n-tiles each)
    nl = n // 256      # natural-layout load tiles (256 rows each)
    ow = min(1024, m)  # psum main tile width (<= 2 banks)
    sw = 2 if nt % 2 == 0 else 1  # n-strips per output DMA

    nc = bacc.Bacc("TRN2", target_bir_lowering=False, debug=False)
    m0 = nc.dram_tensor("mat_0", [bpc, n, h], F32, kind="ExternalInput").ap()
    m1 = nc.dram_tensor("mat_1", [bpc, m, h], F32, kind="ExternalInput").ap()
    # derived inputs (host-precomputed)
    w2c = nc.dram_tensor("w2c", [P, kt], F32, kind="ExternalInput").ap()
    s0t = nc.dram_tensor("s0t", [bpc, P, nt], F32, kind="ExternalInput").ap()
    s1t = nc.dram_tensor("s1t", [bpc, P, m], F32, kind="ExternalInput").ap()
    out = nc.dram_tensor("out", [bpc, n, m], F32, kind="ExternalOutput").ap()

    with TileContext(nc) as tc:
        with (
            tc.tile_pool(name="const", bufs=1) as cpool,
            tc.tile_pool(name="io", bufs=4) as iopool,
            tc.tile_pool(name="opnd", bufs=1) as tpool,
            tc.tile_pool(name="vecs", bufs=2) as vpool,
            tc.tile_pool(name="ob", bufs=2) as obpool,
            tc.tile_pool(name="mpsum", bufs=2, space="PSUM") as mpsum,
            tc.tile_pool(name="tpsum", bufs=2, space="PSUM") as tpsum,
        ):
            identity = cpool.tile([P, P], F32)
            make_identity(nc, identity)
            w2_cols = cpool.tile([P, kt], F32)

            def emit_loads(bi):
                anat, bnat = {}, {}
                for l in range(nl):
                    r0 = 256 * l
                    b_t = iopool.tile([P, 2 * h], F32, tag="bnat", name="b_t")
                    nc.sync.dma_start(
                        out=b_t.rearrange("p (t h) -> p t h", t=2),
                        in_=m1[bi, r0:r0 + 256, :].rearrange(
                            "(t p) h -> p t h", p=P
                        ),
                    )
                    bnat[l] = b_t
                    if bi == 0 and l == 0:
                        # w2_cols only gates the first A-eviction (~10us in);
                        # keep it off the FIFO head so B loads start at t=0
                        nc.sync.dma_start(out=w2_cols, in_=w2c)
                    a_t = iopool.tile([P, 2 * h], F32, tag="anat", name="a_t")
                    nc.sync.dma_start(
                        out=a_t.rearrange("p (t h) -> p t h", t=2),
                        in_=m0[bi, r0:r0 + 256, :].rearrange(
                            "(t p) h -> p t h", p=P
                        ),
                    )
                    anat[l] = a_t
                s0c = vpool.tile([P, nt], F32, tag="s0c", name="s0c")
                nc.sync.dma_start(out=s0c, in_=s0t[bi])
                s1b = vpool.tile([P, m], F32, tag="s1b", name="s1b")
                nc.sync.dma_start(out=s1b, in_=s1t[bi])
                return anat, bnat, s0c, s1b

            def emit_prep(bi, anat, bnat):
                at = [
                    tpool.tile([P, n], F32R, tag=f"at{k}", name=f"at{k}")
                    for k in range(kt)
                ]
                bt = [
                    tpool.tile([P, m], F32R, tag=f"bt{k}", name=f"bt{k}")
                    for k in range(kt)
                ]
                for g in range(ng):
                    for k in range(kt):
                        pt = tpsum.tile([P, 512], F32, tag="tp", name="pt")
                        for j in range(4):
                            t = 4 * g + j
                            src = bnat[t // 2][
                                :, (t % 2) * h + k * P:(t % 2) * h + (k + 1) * P
                            ]
                            nc.tensor.transpose(
                                pt[:, j * P:(j + 1) * P], src, identity
                            )
                        nc.scalar.copy(bt[k][:, g * 512:(g + 1) * 512], pt)
                    for k in range(kt):
                        pt = tpsum.tile([P, 512], F32, tag="tp", name="pt")
                        for j in range(4):
                            t = 4 * g + j
                            src = anat[t // 2][
                                :, (t % 2) * h + k * P:(t % 2) * h + (k + 1) * P
                            ]
                            nc.tensor.transpose(
                                pt[:, j * P:(j + 1) * P], src, identity
                            )
                        nc.scalar.activation(
                            at[k][:, g * 512:(g + 1) * 512],
                            pt,
                            COPY,
                            bias=0.0,
                            scale=w2_cols[:, k:k + 1],
                        )
                return at, bt

            def emit_mains(bi, at, bt, s0c, s1b):
                ob = None
                for t in range(nt):
                    strip = t % sw
                    if strip == 0:
                        ob = obpool.tile([P, sw * m], F32, tag="ob", name="ob", bufs=3)
                    for pc in range(m // ow):
                        mp = mpsum.tile([P, ow], F32, tag="mm", name="mp")
                        for k in range(kt):
                            for mh in range(ow // 512):
                                cm = pc * (ow // 512) + mh
                                nc.tensor.matmul(
                                    mp[:, mh * 512:(mh + 1) * 512],
                                    lhsT=at[k][:, t * P:(t + 1) * P],
                                    rhs=bt[k][:, cm * 512:(cm + 1) * 512],
                                    start=(k == 0),
                                    stop=(k == kt - 1),
                                )
                        nc.vector.scalar_tensor_tensor(
                            out=ob[:, strip * m + pc * ow:strip * m + (pc + 1) * ow],
                            in0=mp,
                            scalar=s0c[:, t:t + 1],
                            in1=s1b[:, pc * ow:(pc + 1) * ow],
                            op0=ADD,
                            op1=ADD,
                        )
                    if bi == bpc - 1 and t >= nt - sw:
                        # final pair: per-strip 1MB stores (shorter tail)
                        nc.sync.dma_start(
                            out=out[bi, t * P:(t + 1) * P, :],
                            in_=ob[:, strip * m:(strip + 1) * m],
                        )
                    elif strip == sw - 1:
                        r0 = (t - sw + 1) * P
                        nc.sync.dma_start(
                            out=out[bi, r0:r0 + sw * P, :].rearrange(
                                "(s p) m -> p s m", p=P
                            ),
                            in_=ob.rearrange("p (s m) -> p s m", s=sw),
                        )

            # software-pipelined emission: next batch's loads go out before
            # this batch's mains so input DMA fills the store-idle window
            la = emit_loads(0)
            prep = emit_prep(0, la[0], la[1])
            vecs = (la[2], la[3])
            for bi in range(1, bpc):
                la_next = emit_loads(bi)
                emit_mains(bi - 1, prep[0], prep[1], vecs[0], vecs[1])
                prep = emit_prep(bi, la_next[0], la_next[1])
                vecs = (la_next[2], la_next[3])
            emit_mains(bpc - 1, prep[0], prep[1], vecs[0], vecs[1])
    nc.compile()
    return nc


_CACHE = {}


def _get_program():
    if "nc" not in _CACHE:
        _CACHE["nc"] = build_program()
    return _CACHE["nc"]


def make_in_maps(inputs, bpc=BPC, n_cores=N_CORES, n=N, m=M, h=H):
    mat_0 = np.ascontiguousarray(np.asarray(inputs["mat_0"], dtype=np.float32))
    mat_1 = np.ascontiguousarray(np.asarray(inputs["mat_1"], dtype=np.float32))
    w = np.asarray(inputs["w"], dtype=np.float32)
    bias = np.asarray(inputs["bias"], dtype=np.float32)
    w0, w1, w2 = w[:h], w[h:2 * h], w[2 * h:]
    kt, nt = h // P, n // P
    # host-side rank-1 epilogue vectors
    s0 = mat_0 @ w0                      # [B, n]
    s1 = mat_1 @ w1 + bias[0]            # [B, m]
    # layouts for direct DMA
    w2c = np.ascontiguousarray(w2.reshape(kt, P).T)          # [P, kt]
    s0t = np.ascontiguousarray(
        s0.reshape(-1, nt, P).transpose(0, 2, 1)             # [B, P, nt]
    )
    s1t = np.ascontiguousarray(
        np.broadcast_to(s1[:, None, :], (s1.shape[0], P, m))  # [B, P, m]
    )
    in_maps = []
    for c in range(n_cores):
        sl = slice(c * bpc, (c + 1) * bpc)
        in_maps.append(
            {
                "mat_0": mat_0[sl],
                "mat_1": mat_1[sl],
                "w2c": w2c,
                "s0t": s0t[sl],
                "s1t": s1t[sl],
            }
        )
    return in_maps


def kernel(**inputs) -> np.ndarray:
    from concourse import bass_utils

    nc = _get_program()
    res = bass_utils.run_bass_kernel_spmd(
        nc, make_in_maps(inputs), core_ids=list(range(N_CORES))
    )
    return np.concatenate(
        [res.results[c]["out"] for c in range(N_CORES)], axis=0
    )
